# revision 1
# baseline (speedup 1.0000x reference)
"""Causal self-attention with ALiBi on 8 trn2 cores.

Sharding: data-parallel over batch (2) x tensor-parallel over head groups (4).
Core c handles batch b = c // 4, head group g = c % 4 (heads 4g..4g+3).
Each core computes qT/kT/v projections for its 4 heads, flash-style causal
attention with ALiBi folded into the score matmul via 2 augmented K rows
(k_aug = [iota_j; ones], q_aug = [slope; -slope*i]), and a partial output
projection.  Host sums the 4 partials per batch and adds bo.

All matmuls run in float32r (~13-bit mantissa, 4x faster than fp32 on PE).
The ALiBi term is exact in f32r: j and slope are exactly representable, and
the -slope*i term is constant per query so softmax cancels its rounding.

The program is fused over t-blocks of 512: for each block, v/q/k projections,
causal attention for that query block, and its slice of the output projection
are emitted together so compute chases the xT DMA wavefront (HBM is shared
with the neighbor core, so the input load is slow relative to compute).
"""

import sys

sys.path.insert(0, "/opt/trn_rl_repo")

import numpy as np

import concourse.bacc as bacc
import concourse.mybir as mybir
import concourse.tile as tile
from concourse.bass import ds, ts
from concourse.bass_utils import run_bass_kernel_spmd

B, T, D, H, DH = 2, 2048, 1024, 16, 64
G = 4            # head groups (tensor-parallel)
HPC = H // G     # heads per core
DG = D // G      # model dims per core (256)
P = 128
N_CORES = 8
NEG = -1.0e30

F32 = mybir.dt.float32
F32R = mybir.dt.float32r
BF16 = mybir.dt.bfloat16
ADD = mybir.AluOpType.add
MULT = mybir.AluOpType.mult
EXP = mybir.ActivationFunctionType.Exp

TRACE = False
LAST_RESULTS = None

_cache = {}


def _build(with_bias: bool):
    nc = bacc.Bacc("TRN2", target_bir_lowering=False, debug=False)

    xT_d = nc.dram_tensor("xT", [D, T], F32, kind="ExternalInput").ap()
    wq_d = nc.dram_tensor("wqT", [D, DG], F32, kind="ExternalInput").ap()
    wk_d = nc.dram_tensor("wkT", [D, DG], F32, kind="ExternalInput").ap()
    wv_d = nc.dram_tensor("wvT", [D, HPC * 65], F32, kind="ExternalInput").ap()
    wo_d = nc.dram_tensor("woT", [DG, D], F32, kind="ExternalInput").ap()
    qaug_d = nc.dram_tensor("qaug", [HPC, 2, T], F32, kind="ExternalInput").ap()
    kaug_d = nc.dram_tensor("kaug", [2, T], F32, kind="ExternalInput").ap()
    ident_d = nc.dram_tensor("ident", [P, P], BF16, kind="ExternalInput").ap()
    maskst_d = nc.dram_tensor("maskst", [P, P], BF16, kind="ExternalInput").ap()
    bvo_d = nc.dram_tensor("bv_ones", [P, HPC * 65], F32, kind="ExternalInput").ap()
    if with_bias:
        bq_d = nc.dram_tensor("bq2", [P, 2], F32, kind="ExternalInput").ap()
        bk_d = nc.dram_tensor("bk2", [P, 2], F32, kind="ExternalInput").ap()
    out_d = nc.dram_tensor("outT", [D, T], F32, kind="ExternalOutput").ap()
    dscr_d = nc.dram_tensor("dscratch", [16, 512], F32).ap()

    with tile.TileContext(nc) as tc:
        with (
            tc.tile_pool(name="big", bufs=1) as big,
            tc.tile_pool(name="xtp", bufs=2) as xtp,
            tc.tile_pool(name="stage", bufs=3) as stage,
            tc.tile_pool(name="expp", bufs=4) as expp,
            tc.tile_pool(name="small", bufs=1) as small,
            tc.tile_pool(name="mm", bufs=2, space="PSUM") as mmp,
            tc.tile_pool(name="pss", bufs=3, space="PSUM") as pssp,
            tc.tile_pool(name="psy", bufs=3, space="PSUM") as psyp,
        ):
            # ---- persistent tiles
            wv = [
                big.tile([P, HPC * 65], F32R, tag=f"wv{i}", name=f"wv{i}")
                for i in range(8)
            ]
            wq = [big.tile([P, DG], F32R, tag=f"wq{i}", name=f"wq{i}") for i in range(8)]
            wk = [big.tile([P, DG], F32R, tag=f"wk{i}", name=f"wk{i}") for i in range(8)]
            wo = [big.tile([P, D], F32R, tag=f"wo{i}", name=f"wo{i}") for i in range(2)]
            qa = [big.tile([P, T], F32R, tag=f"qa{h}", name=f"qa{h}") for h in range(HPC)]
            ka = [big.tile([P, T], F32R, tag=f"ka{h}", name=f"ka{h}") for h in range(HPC)]
            va = [big.tile([P, 16, P], F32R, tag=f"va{h}", name=f"va{h}") for h in range(HPC)]
            yt = [big.tile([P, T], F32R, tag=f"yt{m}", name=f"yt{m}") for m in range(2)]

            # ---- loads. xT streams in [D, 512] column blocks on the GpSimd
            # queue (first block split with Scalar); weights/consts on SP/ACT.
            for i in range(8):
                eng = nc.sync if i % 2 == 0 else nc.scalar
                eng.dma_start(out=wv[i][:], in_=wv_d[ts(i, P), :].bitcast(F32R))
            xtile = [[None] * 8 for _ in range(4)]
            for i in range(8):
                t_ = xtp.tile([P, 512], F32R, tag=f"xt{i}", name=f"x0_{i}")
                eng = nc.gpsimd if i % 2 == 0 else nc.sync
                eng.dma_start(out=t_[:], in_=xT_d[ts(i, P), 0:512].bitcast(F32R))
                xtile[0][i] = t_
            # consts + aug rows (tiny)
            ident_sb = big.tile([P, P], BF16, tag="ident")
            nc.sync.dma_start(out=ident_sb[:], in_=ident_d[:])
            maskst_sb = big.tile([P, P], BF16, tag="maskst")
            nc.sync.dma_start(out=maskst_sb[:], in_=maskst_d[:])
            bvo = big.tile([P, HPC * 65], F32, tag="bvo")
            nc.sync.dma_start(out=bvo[:], in_=bvo_d[:])
            if with_bias:
                bq2 = big.tile([P, 2], F32, tag="bq2")
                nc.sync.dma_start(out=bq2[:], in_=bq_d[:])
                bk2 = big.tile([P, 2], F32, tag="bk2")
                nc.sync.dma_start(out=bk2[:], in_=bk_d[:])
            for h in range(HPC):
                nc.sync.dma_start(out=qa[h][64:66, :], in_=qaug_d[h].bitcast(F32R))
                nc.sync.dma_start(out=ka[h][64:66, :], in_=kaug_d[:].bitcast(F32R))
                # ones column for the in-matmul softmax denominator; the odd
                # head's lands at partition 32 (engine APs need 32-aligned base)
                oc = 64 if h % 2 == 0 else 32
                for ch in range(16):
                    nc.vector.memset(va[h][:, ch, oc : oc + 1].bitcast(F32), 1.0)
            for i in range(8):
                nc.sync.dma_start(out=wq[i][:], in_=wq_d[ts(i, P), :].bitcast(F32R))
                nc.scalar.dma_start(out=wk[i][:], in_=wk_d[ts(i, P), :].bitcast(F32R))
            for i in range(2):
                nc.scalar.dma_start(out=wo[i][:], in_=wo_d[ts(i, P), :].bitcast(F32R))
            # remaining xT blocks: gpsimd only (slot waits may block the queue)
            for tq in range(1, 4):
                for i in range(8):
                    t_ = xtp.tile([P, 512], F32R, tag=f"xt{i}", name=f"x{tq}_{i}")
                    nc.gpsimd.dma_start(
                        out=t_[:], in_=xT_d[ts(i, P), ts(tq, 512)].bitcast(F32R)
                    )
                    xtile[tq][i] = t_

            # ---- projections follow the xT wavefront, one t-block at a time
            for tq in range(4):
                xb = xtile[tq]
                # v projection for this block (natural [t, d] + ones columns)
                for ch in range(4 * tq, 4 * tq + 4):
                    lc = (ch % 4) * P
                    pv = mmp.tile([P, HPC * 65], F32, tag="mm", name=f"pv{ch}")
                    for kc in range(8):
                        nc.tensor.matmul(
                            out=pv[:],
                            lhsT=xb[kc][:, lc : lc + P],
                            rhs=wv[kc][:],
                            start=(kc == 0),
                            stop=(kc == 7),
                        )
                    for h in range(HPC):
                        off = 0 if h % 2 == 0 else 64
                        nc.vector.tensor_tensor(
                            out=va[h][:, ch, off : off + 64],
                            in0=pv[:, h * 65 : h * 65 + 64],
                            in1=bvo[:, h * 65 : h * 65 + 64],
                            op=ADD,
                        )
                # q/k projections for this block into [d', t] layout
                for wt, dst, bias_name in ((wq, qa, "q"), (wk, ka, "k")):
                    bt = (bq2 if bias_name == "q" else bk2) if with_bias else None
                    for mc in range(2):
                        pq = mmp.tile([P, 512], F32, tag="mm", name=f"p{bias_name}{tq}_{mc}")
                        for kc in range(8):
                            nc.tensor.matmul(
                                out=pq[:],
                                lhsT=wt[kc][:, ts(mc, P)],
                                rhs=xb[kc][:],
                                start=(kc == 0),
                                stop=(kc == 7),
                            )
                        h_even, h_odd = 2 * mc, 2 * mc + 1
                        if with_bias:
                            nc.vector.tensor_scalar(
                                out=dst[h_even][0:64, ts(tq, 512)],
                                in0=pq[0:64, :],
                                scalar1=bt[0:64, mc : mc + 1],
                                scalar2=None,
                                op0=ADD,
                            )
                        else:
                            nc.vector.tensor_copy(
                                out=dst[h_even][0:64, ts(tq, 512)], in_=pq[0:64, :]
                            )
                        st = stage.tile([P, 512], F32R, tag="stage", name="st")
                        if with_bias:
                            nc.vector.tensor_scalar(
                                out=st[64:128, :],
                                in0=pq[64:128, :],
                                scalar1=bt[64:128, mc : mc + 1],
                                scalar2=None,
                                op0=ADD,
                            )
                        else:
                            nc.vector.tensor_copy(out=st[64:128, :], in_=pq[64:128, :])
                        nc.sync.dma_start(
                            out=dst[h_odd][0:64, ts(tq, 512)], in_=st[64:128, :]
                        )

            # ---- causal flash attention; the output projection for block qb
            # is emitted after attention for qb+1 so the in-order PE stream
            # never head-of-line blocks on the softmax-normalize chain.
            def emit_attention(qb):
                o = qb * 512
                jmax = qb * 4 + 4
                for pair in range(2):
                    pys = []
                    for h in (2 * pair, 2 * pair + 1):
                        py = psyp.tile([P, 512], F32, tag="psy", name=f"py{qb}_{h}")
                        pys.append(py)
                        pend = None  # software-pipeline: emit AV one chunk behind
                        for jc in range(jmax):
                            r = jc * P - o  # stair offset; diag chunk iff r >= 0
                            ps = pssp.tile(
                                [P, 512], F32, tag="pss", name=f"ps{qb}_{h}_{jc}"
                            )
                            if r < 0:
                                nc.tensor.matmul(
                                    out=ps[:],
                                    lhsT=ka[h][0:66, ts(jc, P)],
                                    rhs=qa[h][0:66, ds(o, 512)],
                                    start=True,
                                    stop=True,
                                )
                                lo = 0
                            else:
                                lo = r
                                nc.tensor.matmul(
                                    out=ps[:, lo:512],
                                    lhsT=ka[h][0:66, ts(jc, P)],
                                    rhs=qa[h][0:66, ds(o + lo, 512 - lo)],
                                    start=True,
                                    stop=False,
                                )
                                # causal stair: ps[:, r:r+128] += I.T @ maskst
                                nc.tensor.matmul(
                                    out=ps[:, lo : lo + P],
                                    lhsT=ident_sb[:],
                                    rhs=maskst_sb[:],
                                    start=False,
                                    stop=True,
                                )
                            ex = expp.tile(
                                [P, 512], F32R, tag="ex", name=f"ex{qb}_{h}_{jc}"
                            )
                            nc.scalar.activation(
                                out=ex[:, lo:512], in_=ps[:, lo:512], func=EXP
                            )
                            if pend is not None:
                                pjc, plo, pex = pend
                                nc.tensor.matmul(
                                    out=py[:, plo:512],
                                    lhsT=va[h][:, pjc, :],
                                    rhs=pex[:, plo:512],
                                    start=(pjc == 0),
                                    stop=False,
                                )
                            pend = (jc, lo, ex)
                        pjc, plo, pex = pend
                        nc.tensor.matmul(
                            out=py[:, plo:512],
                            lhsT=va[h][:, pjc, :],
                            rhs=pex[:, plo:512],
                            start=(pjc == 0),
                            stop=True,
                        )
                    # pair-batched softmax denominators at 32-aligned rows
                    dn = small.tile([P, 512], F32, tag="dn", name=f"dn{qb}_{pair}")
                    for i in range(2):
                        dr = 64 if i == 0 else 32
                        nc.vector.tensor_copy(
                            out=dn[i * 32 : i * 32 + 1, :], in_=pys[i][dr : dr + 1, :]
                        )
                    dn2 = small.tile([P, 512], F32, tag="dn2", name=f"dn2{qb}_{pair}")
                    nc.vector.reciprocal(out=dn2[0:64, :], in_=dn[0:64, :])
                    for i in range(2):
                        h = 2 * pair + i
                        rowbase = i * 64
                        idx = qb * HPC + h
                        nc.sync.dma_start(
                            out=dscr_d[idx : idx + 1, :],
                            in_=dn2[i * 32 : i * 32 + 1, :],
                        )
                        rb = small.tile([P, 512], F32, tag="rb", name=f"rb{qb}_{h}")
                        nc.sync.dma_start(
                            out=rb[rowbase : rowbase + 64, :],
                            in_=dscr_d[idx : idx + 1, :].to_broadcast((64, 512)),
                        )
                        nc.vector.tensor_tensor(
                            out=yt[pair][rowbase : rowbase + 64, ds(o, 512)],
                            in0=pys[i][rowbase : rowbase + 64, :],
                            in1=rb[rowbase : rowbase + 64, :],
                            op=MULT,
                        )

            def emit_outproj(qb):
                for ec in range(8):
                    po = psyp.tile([P, 512], F32, tag="psy", name=f"po{qb}_{ec}")
                    for k2 in range(2):
                        nc.tensor.matmul(
                            out=po[:],
                            lhsT=wo[k2][:, ts(ec, P)],
                            rhs=yt[k2][:, ts(qb, 512)],
                            start=(k2 == 0),
                            stop=(k2 == 1),
                        )
                    ob = stage.tile([P, 512], F32, tag="stage", name="ob")
                    nc.vector.tensor_copy(out=ob[:], in_=po[:])
                    nc.sync.dma_start(out=out_d[ts(ec, P), ts(qb, 512)], in_=ob[:])

            for qb in range(4):
                emit_attention(qb)
                if qb > 0:
                    emit_outproj(qb - 1)
            emit_outproj(3)

    nc.compile()
    return nc


def _get_nc(with_bias: bool):
    if with_bias not in _cache:
        _cache[with_bias] = _build(with_bias)
    return _cache[with_bias]


def kernel(x, freqs_cis, Wq, bq, Wkv, bkv, Wo, bo, **_unused):
    x = np.asarray(x, np.float32)
    Wq = np.asarray(Wq, np.float32)
    bq = np.asarray(bq, np.float32)
    Wkv = np.asarray(Wkv, np.float32)
    bkv = np.asarray(bkv, np.float32)
    Wo = np.asarray(Wo, np.float32)
    bo = np.asarray(bo, np.float32)

    with_bias = bool(np.any(bq) or np.any(bkv))
    nc = _get_nc(with_bias)

    scale = 1.0 / np.sqrt(DH)
    iota = np.arange(T, dtype=np.float32)

    # causal stair (applied via identity-matmul accumulation into PSUM):
    # maskst[p, m] = -1e30 where m < p (j = chunk base + p is in the future)
    import ml_dtypes
    mm = np.arange(P, dtype=np.float32)
    maskst = np.where(mm[None, :] < mm[:, None], NEG, 0.0).astype(ml_dtypes.bfloat16)
    ident = np.eye(P, dtype=ml_dtypes.bfloat16)

    kaug = np.stack([iota, np.ones(T, np.float32)])  # [2, T]

    xT = [np.ascontiguousarray(x[b].T) for b in range(B)]  # [D, T]

    in_maps = []
    for c in range(N_CORES):
        b, g = divmod(c, G)
        rows = slice(g * DG, (g + 1) * DG)
        wqT = np.ascontiguousarray((Wq[rows] * scale).T)          # [D, DG]
        wkT = np.ascontiguousarray(Wkv[0:D][rows].T)              # [D, DG]
        wv_g = Wkv[D : 2 * D][rows]                               # [DG, D]
        bv_g = bkv[D : 2 * D][rows]                               # [DG]
        # v weights with one zero column per head block of 65
        wvT = np.zeros((D, HPC * 65), np.float32)
        bvo = np.zeros((P, HPC * 65), np.float32)
        for h in range(HPC):
            wvT[:, h * 65 : h * 65 + 64] = wv_g[h * 64 : (h + 1) * 64].T
            bvo[:, h * 65 : h * 65 + 64] = bv_g[h * 64 : (h + 1) * 64][None, :]
        woT = np.ascontiguousarray(Wo[:, rows].T)                 # [DG, D]
        qaug = np.zeros((HPC, 2, T), np.float32)
        for h in range(HPC):
            slope = (g * HPC + h + 1) / H
            qaug[h, 0, :] = slope
            qaug[h, 1, :] = -slope * iota
        m = {
            "xT": xT[b],
            "wqT": wqT,
            "wkT": wkT,
            "wvT": wvT,
            "woT": woT,
            "qaug": qaug,
            "kaug": kaug,
            "ident": ident,
            "maskst": maskst,
            "bv_ones": bvo,
        }
        if with_bias:
            m["bq2"] = np.ascontiguousarray((bq[rows] * scale).reshape(2, P).T)
            m["bk2"] = np.ascontiguousarray(bkv[0:D][rows].reshape(2, P).T)
        in_maps.append(m)

    res = run_bass_kernel_spmd(nc, in_maps, list(range(N_CORES)), trace=TRACE)
    global LAST_RESULTS
    LAST_RESULTS = res

    out = np.empty((B, T, D), np.float32)
    for b in range(B):
        acc = res.results[b * G]["outT"].copy()
        for g in range(1, G):
            acc += res.results[b * G + g]["outT"]
        out[b] = acc.T + bo[None, :]
    return out



# revision 14
# speedup vs baseline: 1.1071x; 1.1071x over previous
"""Causal self-attention with ALiBi on 8 trn2 cores.

Sharding: data-parallel over batch (2) x tensor-parallel over head groups (4).
Core c handles batch b = c // 4, head group g = c % 4 (heads 4g..4g+3).

v2 rewrite vs the f32r baseline:
  * everything that moves is bf16 (x, weights, q/k/v, exp weights, yt, out):
    halves HBM traffic and doubles effective DVE rate; PE runs 1 cycle/row
    for bf16 just like f32r.
  * ALiBi rides in the score matmul via 3 bf16-exact aug rows
    (k_aug = [j//256, j%256, 1], q_aug = [256*s, s, -s*i - C]).  The -s*i-C
    row is bf16-rounded but constant per query column, so softmax cancels it.
  * C = max(0, 127*s - 45) is a per-head exponent shift that keeps
    exp() finite on the (pre-mask) upper-triangle cells of diagonal chunks,
    letting the causal mask run as a cheap post-exp DVE multiply by a 0/1
    stair instead of a -1e30 stair matmul on the PE.
  * softmax denominators broadcast across partitions with one tiny PE
    matmul (sel row x recip row) instead of the SBUF->DRAM->SBUF bounce.
  * exp is batched two 128-key chunks per activation instruction.
  * projection matmul groups for t-block tq+1 are interleaved into the
    attention head loops of query block qb=tq, so the PE fills exp-wait
    gaps and never queues behind the x DMA wavefront.
"""

import sys

sys.path.insert(0, "/opt/trn_rl_repo")

import numpy as np

import concourse.bacc as bacc
import concourse.mybir as mybir
import concourse.tile as tile
from concourse.bass import ds, ts
from concourse.bass_utils import run_bass_kernel_spmd

B, T, D, H, DH = 2, 2048, 1024, 16, 64
G = 4            # head groups (tensor-parallel)
HPC = H // G     # heads per core
DG = D // G      # model dims per core (256)
P = 128
N_CORES = 8

F32 = mybir.dt.float32
F32R = mybir.dt.float32r
BF16 = mybir.dt.bfloat16
ADD = mybir.AluOpType.add
MULT = mybir.AluOpType.mult
EXP = mybir.ActivationFunctionType.Exp

TRACE = False
LAST_RESULTS = None

_cache = {}


def _build(with_bias: bool):
    nc = bacc.Bacc("TRN2", target_bir_lowering=False, debug=False)

    xT_d = nc.dram_tensor("xT", [D, T], BF16, kind="ExternalInput").ap()
    wq_d = nc.dram_tensor("wqT", [D, DG], BF16, kind="ExternalInput").ap()
    wk_d = nc.dram_tensor("wkT", [D, DG], BF16, kind="ExternalInput").ap()
    wv_d = nc.dram_tensor("wvT", [D, DG], BF16, kind="ExternalInput").ap()
    wo_d = nc.dram_tensor("woT", [DG, D], BF16, kind="ExternalInput").ap()
    qaug_d = nc.dram_tensor("qaug", [HPC, 5, T], BF16, kind="ExternalInput").ap()
    kaug_d = nc.dram_tensor("kaug", [5, T], BF16, kind="ExternalInput").ap()
    stair_d = nc.dram_tensor("stair01", [P, P], BF16, kind="ExternalInput").ap()
    sel_d = nc.dram_tensor("sel", [P, P], F32, kind="ExternalInput").ap()
    if with_bias:
        bq_d = nc.dram_tensor("bq2", [P, 2], F32, kind="ExternalInput").ap()
        bk_d = nc.dram_tensor("bk2", [P, 2], F32, kind="ExternalInput").ap()
        bvo_d = nc.dram_tensor("bvo", [P, DG], F32, kind="ExternalInput").ap()
    out_d = nc.dram_tensor("outT", [D, T], BF16, kind="ExternalOutput").ap()

    with tile.TileContext(nc) as tc:
        with (
            tc.tile_pool(name="big", bufs=1) as big,
            tc.tile_pool(name="xtp", bufs=4) as xtp,
            tc.tile_pool(name="stage", bufs=3) as stage,
            tc.tile_pool(name="expp", bufs=3) as expp,
            tc.tile_pool(name="mm", bufs=2, space="PSUM") as mmp,
            tc.tile_pool(name="pss", bufs=2, space="PSUM") as pssp,
            tc.tile_pool(name="pyp", bufs=2, space="PSUM") as pyp,
        ):
            # ---- persistent tiles
            wq = [big.tile([P, DG], BF16, tag=f"wq{i}", name=f"wq{i}") for i in range(8)]
            wk = [big.tile([P, DG], BF16, tag=f"wk{i}", name=f"wk{i}") for i in range(8)]
            wv = [big.tile([P, DG], BF16, tag=f"wv{i}", name=f"wv{i}") for i in range(8)]
            wo = [big.tile([P, D], BF16, tag=f"wo{i}", name=f"wo{i}") for i in range(2)]
            qa = [big.tile([P, T], BF16, tag=f"qa{h}", name=f"qa{h}") for h in range(HPC)]
            ka = [big.tile([P, T], BF16, tag=f"ka{h}", name=f"ka{h}") for h in range(HPC)]
            va = [big.tile([P, 16, P], BF16, tag=f"va{h}", name=f"va{h}") for h in range(HPC)]
            yt = [big.tile([P, T], BF16, tag=f"yt{m}", name=f"yt{m}") for m in range(2)]
            sel_sb = big.tile([P, P], F32R, tag="sel")
            stair_sb = big.tile([P, P], BF16, tag="stair")
            rden = big.tile([P, 512], F32R, tag="rden")
            # rows 0..63 are contracted by the denominator-broadcast matmul
            # (sel is zero there) — they must hold zeros, not garbage.
            nc.vector.memset(rden[0:64, :].bitcast(F32), 0.0)

            # ---- loads.  x streams in [D, 512] column blocks on the GpSimd
            # queue; weights/consts on SP/ACT.
            xtile = [[None] * 8 for _ in range(4)]
            for i in range(8):
                t_ = xtp.tile([P, 512], BF16, tag=f"xt{i}", name=f"x0_{i}")
                eng = nc.gpsimd if i % 2 == 0 else nc.sync
                eng.dma_start(out=t_[:], in_=xT_d[ts(i, P), 0:512])
                xtile[0][i] = t_
            for i in range(8):
                nc.sync.dma_start(out=wq[i][:], in_=wq_d[ts(i, P), :])
                nc.scalar.dma_start(out=wk[i][:], in_=wk_d[ts(i, P), :])
            nc.sync.dma_start(out=sel_sb[:], in_=sel_d[:].bitcast(F32R))
            nc.sync.dma_start(out=stair_sb[:], in_=stair_d[:])
            for h in range(HPC):
                nc.sync.dma_start(out=qa[h][64:69, :], in_=qaug_d[h])
                nc.sync.dma_start(out=ka[h][64:69, :], in_=kaug_d[:])
                # ones column for the in-matmul softmax denominator; odd heads
                # put v at cols 64..127 so their ones column sits at col 32.
                # The unused lhsT columns must be zero, not garbage.
                oc = 64 if h % 2 == 0 else 32
                nc.vector.memset(va[h][:], 0.0)
                nc.vector.memset(va[h][:, :, oc : oc + 1], 1.0)

            if with_bias:
                bq2 = big.tile([P, 2], F32, tag="bq2")
                nc.sync.dma_start(out=bq2[:], in_=bq_d[:])
                bk2 = big.tile([P, 2], F32, tag="bk2")
                nc.sync.dma_start(out=bk2[:], in_=bk_d[:])
                bvo = big.tile([P, DG], F32, tag="bvo")
                nc.sync.dma_start(out=bvo[:], in_=bvo_d[:])
            for i in range(8):
                eng = nc.sync if i % 2 == 0 else nc.scalar
                eng.dma_start(out=wv[i][:], in_=wv_d[ts(i, P), :])
            for i in range(2):
                nc.scalar.dma_start(out=wo[i][:], in_=wo_d[ts(i, P), :])
            for tq in range(1, 4):
                for i in range(8):
                    t_ = xtp.tile([P, 512], BF16, tag=f"xt{i}", name=f"x{tq}_{i}")
                    nc.gpsimd.dma_start(out=t_[:], in_=xT_d[ts(i, P), ts(tq, 512)])
                    xtile[tq][i] = t_

            # ---- projection groups; one group = one PSUM tile (8 matmuls).
            # proj_groups(tq) returns 8 closures so attention can interleave
            # them between its head loops.
            def emit_qk_group(tq, wt, dst, bias_t, mc):
                xb = xtile[tq]
                pq = mmp.tile([P, 512], F32, tag="mm", name=f"pq{tq}_{mc}")
                for kc in range(8):
                    nc.tensor.matmul(
                        out=pq[:],
                        lhsT=wt[kc][:, ts(mc, P)],
                        rhs=xb[kc][:],
                        start=(kc == 0),
                        stop=(kc == 7),
                    )
                h_e, h_o = 2 * mc, 2 * mc + 1
                if with_bias:
                    nc.vector.tensor_scalar(
                        out=dst[h_e][0:64, ts(tq, 512)],
                        in0=pq[0:64, :],
                        scalar1=bias_t[0:64, mc : mc + 1],
                        scalar2=None,
                        op0=ADD,
                    )
                else:
                    nc.vector.tensor_copy(out=dst[h_e][0:64, ts(tq, 512)], in_=pq[0:64, :])
                st = stage.tile([P, 512], BF16, tag="st", name=f"st{tq}_{mc}")
                if with_bias:
                    nc.vector.tensor_scalar(
                        out=st[64:128, :],
                        in0=pq[64:128, :],
                        scalar1=bias_t[64:128, mc : mc + 1],
                        scalar2=None,
                        op0=ADD,
                    )
                else:
                    nc.vector.tensor_copy(out=st[64:128, :], in_=pq[64:128, :])
                nc.sync.dma_start(out=dst[h_o][0:64, ts(tq, 512)], in_=st[64:128, :])

            def emit_v_group(tq, ch):
                xb = xtile[tq]
                lc = (ch % 4) * P
                pv = mmp.tile([P, 512], F32, tag="mm", name=f"pv{ch}")
                for kc in range(8):
                    nc.tensor.matmul(
                        out=pv[:, 0:DG],
                        lhsT=xb[kc][:, lc : lc + P],
                        rhs=wv[kc][:],
                        start=(kc == 0),
                        stop=(kc == 7),
                    )
                for h in range(HPC):
                    off = 0 if h % 2 == 0 else 64
                    if with_bias:
                        nc.vector.tensor_tensor(
                            out=va[h][:, ch, off : off + 64],
                            in0=pv[:, h * 64 : h * 64 + 64],
                            in1=bvo[:, h * 64 : h * 64 + 64],
                            op=ADD,
                        )
                    else:
                        nc.vector.tensor_copy(
                            out=va[h][:, ch, off : off + 64],
                            in_=pv[:, h * 64 : h * 64 + 64],
                        )

            def proj_groups(tq):
                gs = []
                bqt = bq2 if with_bias else None
                bkt = bk2 if with_bias else None
                for mc in range(2):
                    gs.append(lambda tq=tq, mc=mc, b=bqt: emit_qk_group(tq, wq, qa, b, mc))
                for mc in range(2):
                    gs.append(lambda tq=tq, mc=mc, b=bkt: emit_qk_group(tq, wk, ka, b, mc))
                for ch in range(4 * tq, 4 * tq + 4):
                    gs.append(lambda tq=tq, ch=ch: emit_v_group(tq, ch))
                return gs

            # ---- causal flash attention for query block qb, interleaving
            # the projection groups of t-block qb+1 and the output projection
            # of block qb-1.
            def emit_av_group(py, h, info, ex, last):
                for jj, (jc, lo) in enumerate(info):
                    nc.tensor.matmul(
                        out=py[:, lo:512],
                        lhsT=va[h][:, jc, :],
                        rhs=ex[:, jj * 512 + lo : (jj + 1) * 512],
                        start=(jc == 0),
                        stop=(last and jj == len(info) - 1),
                    )

            def emit_outproj_half(qb, half):
                for ec in range(4 * half, 4 * half + 4):
                    po = mmp.tile([P, 512], F32, tag="mm", name=f"po{qb}_{ec}")
                    for k2 in range(2):
                        nc.tensor.matmul(
                            out=po[:],
                            lhsT=wo[k2][:, ts(ec, P)],
                            rhs=yt[k2][:, ts(qb, 512)],
                            start=(k2 == 0),
                            stop=(k2 == 1),
                        )
                    ob = stage.tile([P, 512], BF16, tag="ob", name=f"ob{qb}_{ec}")
                    nc.vector.tensor_copy(out=ob[:], in_=po[:])
                    nc.sync.dma_start(out=out_d[ts(ec, P), ts(qb, 512)], in_=ob[:])

            def emit_attention(qb):
                o = qb * 512
                ngroups = 2 * qb + 2
                pg = proj_groups(qb + 1) if qb < 3 else [None] * 8
                for pair in range(2):
                    pys = []
                    for h in (2 * pair, 2 * pair + 1):
                        py = pyp.tile([P, 512], F32, tag="py", name=f"py{qb}_{h}")
                        pys.append(py)
                        pend = None  # software-pipeline: AV one group behind
                        for g in range(ngroups):
                            ps2 = pssp.tile(
                                [P, 1024], F32, tag="pss", name=f"ps{qb}_{h}_{g}"
                            )
                            info = []
                            for jj in range(2):
                                jc = 2 * g + jj
                                r = jc * P - o
                                lo = max(r, 0)
                                nc.tensor.matmul(
                                    out=ps2[:, jj * 512 + lo : (jj + 1) * 512],
                                    lhsT=ka[h][0:69, ts(jc, P)],
                                    rhs=qa[h][0:69, ds(o + lo, 512 - lo)],
                                    start=True,
                                    stop=True,
                                )
                                info.append((jc, lo))
                            ex = expp.tile(
                                [P, 1024], BF16, tag="ex", name=f"ex{qb}_{h}_{g}"
                            )
                            if info[0][1] == 0 and info[1][1] == 0:
                                nc.scalar.activation(out=ex[:], in_=ps2[:], func=EXP)
                            else:
                                for jj, (jc, lo) in enumerate(info):
                                    nc.scalar.activation(
                                        out=ex[:, jj * 512 + lo : (jj + 1) * 512],
                                        in_=ps2[:, jj * 512 + lo : (jj + 1) * 512],
                                        func=EXP,
                                    )
                            for jj, (jc, lo) in enumerate(info):
                                if jc * P - o >= 0:  # diagonal chunk: 0/1 stair
                                    nc.vector.tensor_tensor(
                                        out=ex[:, jj * 512 + lo : jj * 512 + lo + P],
                                        in0=ex[:, jj * 512 + lo : jj * 512 + lo + P],
                                        in1=stair_sb[:],
                                        op=MULT,
                                    )
                            if pend is not None:
                                emit_av_group(py, h, pend[0], pend[1], False)
                            pend = (info, ex)
                        emit_av_group(py, h, pend[0], pend[1], True)
                        # two projection groups of the next t-block per head
                        gidx = 2 * (2 * pair + (h % 2))
                        for gi in (gidx, gidx + 1):
                            if pg[gi] is not None:
                                pg[gi]()
                    # ---- pair normalize: 1/den rows -> PE broadcast -> yt
                    with nc.allow_low_precision(reason="1/den in f32r, 2^-13 rel"):
                        nc.vector.reciprocal(out=rden[64:65, :], in_=pys[0][64:65, :])
                        nc.vector.reciprocal(out=rden[32:33, :], in_=pys[1][32:33, :])
                    # matmul operands must start at partition 0 on HW; sel is
                    # zero except rows 64 (-> bc rows 0..63) and 32 (-> 64..127),
                    # so one base-0 contraction over [0:65) does both heads.
                    bc = mmp.tile([P, 512], F32, tag="mm", name=f"bc{qb}_{pair}")
                    nc.tensor.matmul(
                        out=bc[:],
                        lhsT=sel_sb[0:65, :],
                        rhs=rden[0:65, :],
                        start=True,
                        stop=True,
                    )
                    # DVE has a single PSUM read port, so the broadcast must
                    # bounce through SBUF before the py (PSUM) multiply.
                    bcs = stage.tile([P, 512], F32, tag="bcs", name=f"bcs{qb}_{pair}")
                    nc.vector.tensor_copy(out=bcs[:], in_=bc[:])
                    for i in range(2):
                        rb = i * 64
                        nc.vector.tensor_tensor(
                            out=yt[pair][rb : rb + 64, ds(o, 512)],
                            in0=pys[i][rb : rb + 64, :],
                            in1=bcs[rb : rb + 64, :],
                            op=MULT,
                        )
                    if qb > 0:
                        emit_outproj_half(qb - 1, pair)

            for g in proj_groups(0):
                g()
            for qb in range(4):
                emit_attention(qb)
            emit_outproj_half(3, 0)
            emit_outproj_half(3, 1)

    nc.compile()
    return nc


def _get_nc(with_bias: bool):
    if with_bias not in _cache:
        _cache[with_bias] = _build(with_bias)
    return _cache[with_bias]


def kernel(x, freqs_cis, Wq, bq, Wkv, bkv, Wo, bo, **_unused):
    import ml_dtypes

    BF = ml_dtypes.bfloat16
    x = np.asarray(x, np.float32)
    Wq = np.asarray(Wq, np.float32)
    bq = np.asarray(bq, np.float32)
    Wkv = np.asarray(Wkv, np.float32)
    bkv = np.asarray(bkv, np.float32)
    Wo = np.asarray(Wo, np.float32)
    bo = np.asarray(bo, np.float32)

    with_bias = bool(np.any(bq) or np.any(bkv))
    nc = _get_nc(with_bias)

    scale = 1.0 / np.sqrt(DH)
    iota = np.arange(T, dtype=np.float32)

    # aug rows: s*j = 256s*(j//256) + s*(j%256), both factors bf16-exact.
    # all aug factors are bf16-exact (<= 8 significand bits); only the
    # -s*(i%16)-C row rounds (+-0.25), and that is constant per query so
    # softmax cancels it.
    i_hi = np.floor(iota / 256.0)
    i_md = np.floor(iota / 16.0) - 16.0 * i_hi
    i_lo = iota - 256.0 * i_hi - 16.0 * i_md
    kaug = np.stack([i_hi, iota - 256.0 * i_hi, np.ones(T), np.ones(T), np.ones(T)]).astype(BF)
    mm_ = np.arange(P, dtype=np.float32)
    stair01 = (mm_[None, :] >= mm_[:, None]).astype(BF)  # keep where m >= p
    sel = np.zeros((P, P), np.float32)
    sel[64, 0:64] = 1.0  # even head recips -> partitions 0..63
    sel[32, 64:128] = 1.0  # odd head recips -> partitions 64..127

    xT = [np.ascontiguousarray(x[b].T).astype(BF) for b in range(B)]

    in_maps = []
    for c in range(N_CORES):
        b, g = divmod(c, G)
        rows = slice(g * DG, (g + 1) * DG)
        wqT = np.ascontiguousarray((Wq[rows] * scale).T).astype(BF)
        wkT = np.ascontiguousarray(Wkv[0:D][rows].T).astype(BF)
        wvT = np.ascontiguousarray(Wkv[D : 2 * D][rows].T).astype(BF)
        woT = np.ascontiguousarray(Wo[:, rows].T).astype(BF)
        qaug = np.zeros((HPC, 5, T), np.float32)
        for h in range(HPC):
            s = (g * HPC + h + 1) / H
            C = max(0.0, 127.0 * s - 50.0)
            qaug[h, 0, :] = 256.0 * s
            qaug[h, 1, :] = s
            qaug[h, 2, :] = -256.0 * s * i_hi
            qaug[h, 3, :] = -16.0 * s * i_md
            qaug[h, 4, :] = -s * i_lo - C
        m = {
            "xT": xT[b],
            "wqT": wqT,
            "wkT": wkT,
            "wvT": wvT,
            "woT": woT,
            "qaug": qaug.astype(BF),
            "kaug": kaug,
            "stair01": stair01,
            "sel": sel,
        }
        if with_bias:
            m["bq2"] = np.ascontiguousarray((bq[rows] * scale).reshape(2, P).T)
            m["bk2"] = np.ascontiguousarray(bkv[0:D][rows].reshape(2, P).T)
            bv = bkv[D : 2 * D][rows]
            m["bvo"] = np.broadcast_to(bv[None, :], (P, DG)).copy()
        in_maps.append(m)

    res = run_bass_kernel_spmd(nc, in_maps, list(range(N_CORES)), trace=TRACE)
    global LAST_RESULTS
    LAST_RESULTS = res

    out = np.empty((B, T, D), np.float32)
    for b in range(B):
        acc = res.results[b * G]["outT"].astype(np.float32)
        for g in range(1, G):
            acc += res.results[b * G + g]["outT"].astype(np.float32)
        out[b] = acc.T + bo[None, :]
    return out
